# revision 1
# baseline (speedup 1.0000x reference)
"""BoundaryLoss TRN2 kernel — 8-core data-parallel (b x H-half).

Math (exact restructuring of the reference):
  p = sigmoid(inputs); mask_p = (p != 0) = 1 everywhere for this data regime
  (|logits| < 40), so erode6(mask_p) = E = interior indicator (0 on any
  volume face, 1 inside). boundary_inputs = p0 + p1 - 2E.
  Interior voxels: p0+p1-2 < 0  =>  bi = clip(.) = EPS exactly, so the
  per-voxel loss is affine in bt = boundary_targets:
      f_int(bt) = -(bt*log(EPS) + (1-bt)*log1p(-EPS))
  Face voxels (d in {0,127} or h in {0,191} or w in {0,191}):
      bi = clip(p0+p1, EPS, 1-EPS),  bt = t0 + t1  (erosion of targets is 0
      at faces), full BCE evaluated directly.
  Total = sum_int f_int(bt) + sum_faces f(bt, bi); the only dense device
  work is the 6-connectivity erosion of the two target channels and exact
  popcount-style sums of the boundary map.

Device pipeline per core (b, H-half), SPMD on 8 NeuronCores:
  - targets slab int32 [2, 128, 98*192] (1-row halos, zeros at volume edge)
    DMA-cast to int8; u = t0 | (t1 << 3) packs both channels per byte.
  - erosion via pure-bitwise AND of 7 taps (w+-1: byte-shifted SBUF-SBUF DMA
    copies; d+-1: partition-shifted DMA copies; h+-1: in-tile views).
  - B = u ^ e  -> bytes bt0 + 8*bt1.
  - Sums via ScalarE activation(Copy) accum_out (fp32-exact integer sums).
  - Small host-gathered face arrays get the full BCE on device.
"""
import sys
sys.path.insert(0, "/opt/trn_rl_repo")

import numpy as np

B_DIM, C_DIM, D_DIM, H_DIM, W_DIM = 4, 2, 128, 192, 192
N_CORES = 8
HH = H_DIM // 2            # 96 own rows per core
SLAB_ROWS = HH + 2         # with halo
ROW_B = W_DIM              # 192 bytes per row (int8)
CHUNK_ROWS = 32            # own rows per chunk
N_CHUNKS = HH // CHUNK_ROWS
OWN_B = CHUNK_ROWS * ROW_B           # 6144 bytes per chunk (own window)
OWN_W = OWN_B // 4                   # 1536 int32 words
LOAD_ROWS = CHUNK_ROWS + 2           # 34
LOAD_B = LOAD_ROWS * ROW_B           # 6528
FACE_N = 2 * HH * W_DIM + (D_DIM - 2) * W_DIM + (D_DIM - 2) * (HH - 1) * 2  # 84996
FACE_F = 672                         # per-partition face elems (128*672 = 86016)
FACE_PAD = 128 * FACE_F - FACE_N
EPS = 1e-7
N_MEAN = B_DIM * D_DIM * H_DIM * W_DIM  # 18874368
OUT_COLS = 16

_compiled = None


def _build_bass():
    import concourse.bacc as bacc
    import concourse.tile as tile
    from concourse import mybir
    from contextlib import ExitStack

    dt = mybir.dt
    Alu = mybir.AluOpType
    P = 128

    nc = bacc.Bacc("TRN2", target_bir_lowering=False, debug=False,
                   num_devices=N_CORES)
    tslab = nc.declare_dram_parameter(
        "tslab", [C_DIM, P, SLAB_ROWS * ROW_B], dt.int32, isOutput=False)
    xf = nc.declare_dram_parameter(
        "xf", [C_DIM, P, FACE_F], dt.float32, isOutput=False)
    btf = nc.declare_dram_parameter(
        "btf", [P, FACE_F], dt.float32, isOutput=False)
    out = nc.declare_dram_parameter(
        "out", [P, OUT_COLS], dt.float32, isOutput=True)

    import os as _os
    _bufs = int(_os.environ.get("BDL_BUFS", "2"))
    with tile.TileContext(nc) as tc, ExitStack() as ctx:
        io_pool = ctx.enter_context(tc.tile_pool(name="io", bufs=_bufs))
        sh_pool = ctx.enter_context(tc.tile_pool(name="sh", bufs=_bufs))
        small = ctx.enter_context(tc.tile_pool(name="small", bufs=1))

        sc3 = small.tile([P, 1], dt.int32)
        nc.vector.memset(sc3[:], 3)
        zrow = small.tile([1, OWN_B], dt.int8)
        nc.vector.memset(zrow[:], 0)

        stage = small.tile([P, OUT_COLS], dt.float32)
        nc.vector.memset(stage[:], 0.0)

        for ck in range(N_CHUNKS):
            r0 = ck * CHUNK_ROWS           # slab row of chunk halo start
            lo_b = r0 * ROW_B              # load window byte offset

            q0 = io_pool.tile([P, LOAD_B], dt.int8, tag="q0")
            q1 = io_pool.tile([P, LOAD_B], dt.int8, tag="q1")
            nc.gpsimd.dma_start(q0[:], tslab[0, :, lo_b:lo_b + LOAD_B])
            nc.gpsimd.dma_start(q1[:], tslab[1, :, lo_b:lo_b + LOAD_B])

            u = io_pool.tile([P, LOAD_B], dt.int8, tag="u")
            uw = u[:].bitcast(dt.int32)
            nc.vector.scalar_tensor_tensor(
                uw, q1[:].bitcast(dt.int32), sc3[:, 0:1], q0[:].bitcast(dt.int32),
                op0=Alu.logical_shift_left, op1=Alu.bitwise_or)

            # shifted copies of the own window (bytes [192, 6336))
            uw1 = sh_pool.tile([P, OWN_B], dt.int8, tag="uw1")
            uwm1 = sh_pool.tile([P, OWN_B], dt.int8, tag="uwm1")
            ud1 = sh_pool.tile([P, OWN_B], dt.int8, tag="ud1")
            udm1 = sh_pool.tile([P, OWN_B], dt.int8, tag="udm1")
            nc.sync.dma_start(uw1[:], u[:, ROW_B - 1:ROW_B - 1 + OWN_B])
            nc.sync.dma_start(uwm1[:], u[:, ROW_B + 1:ROW_B + 1 + OWN_B])
            nc.sync.dma_start(ud1[0:P - 1, :], u[1:P, ROW_B:ROW_B + OWN_B])
            nc.sync.dma_start(udm1[1:P, :], u[0:P - 1, ROW_B:ROW_B + OWN_B])
            # zero out-of-volume taps
            uw1_3d = uw1[:].rearrange("p (r w) -> p r w", w=ROW_B)
            uwm1_3d = uwm1[:].rearrange("p (r w) -> p r w", w=ROW_B)
            nc.vector.memset(uw1_3d[:, :, 0:1], 0)
            nc.vector.memset(uwm1_3d[:, :, ROW_B - 1:ROW_B], 0)
            nc.sync.dma_start(ud1[P - 1:P, :], zrow[:])
            nc.sync.dma_start(udm1[0:1, :], zrow[:])

            # erosion: e = uo & all 6 neighbor taps (pure bitwise, exact)
            uo = uw[:, 48:48 + OWN_W]              # own window (words)
            uh1 = uw[:, 96:96 + OWN_W]             # h+1 view
            uhm1 = uw[:, 0:OWN_W]                  # h-1 view
            e_t = sh_pool.tile([P, OWN_B], dt.int8, tag="e")
            ew = e_t[:].bitcast(dt.int32)
            nc.vector.tensor_tensor(ew, uo, uh1, op=Alu.bitwise_and)
            nc.vector.tensor_tensor(ew, ew, uhm1, op=Alu.bitwise_and)
            nc.vector.tensor_tensor(ew, ew, uw1[:].bitcast(dt.int32), op=Alu.bitwise_and)
            nc.vector.tensor_tensor(ew, ew, uwm1[:].bitcast(dt.int32), op=Alu.bitwise_and)
            nc.vector.tensor_tensor(ew, ew, ud1[:].bitcast(dt.int32), op=Alu.bitwise_and)
            nc.vector.tensor_tensor(ew, ew, udm1[:].bitcast(dt.int32), op=Alu.bitwise_and)

            # B = u ^ e : bytes = bt0 + 8*bt1
            B_t = sh_pool.tile([P, OWN_B], dt.int8, tag="B")
            Bw = B_t[:].bitcast(dt.int32)
            nc.vector.tensor_tensor(Bw, uo, ew, op=Alu.bitwise_xor)

            # sums: col ck = sum(B bytes) = Sbt0 + 8*Sbt1 ; col 3+ck = Sbt1
            m1 = sh_pool.tile([P, OWN_B], dt.int8, tag="m1")
            nc.vector.tensor_scalar(
                m1[:].bitcast(dt.int32), Bw, 3, 0x01010101,
                op0=Alu.logical_shift_right, op1=Alu.bitwise_and)
            junk = sh_pool.tile([P, OWN_B], dt.int8, tag="junk")
            accB = small.tile([P, 1], dt.float32, tag=f"accB{ck}")
            acc1 = small.tile([P, 1], dt.float32, tag=f"acc1{ck}")
            nc.scalar.activation(junk[:], B_t[:],
                                 mybir.ActivationFunctionType.Copy,
                                 accum_out=accB[:])
            nc.scalar.activation(junk[:], m1[:],
                                 mybir.ActivationFunctionType.Copy,
                                 accum_out=acc1[:])
            nc.vector.tensor_copy(stage[:, ck:ck + 1], accB[:])
            nc.vector.tensor_copy(stage[:, 3 + ck:4 + ck], acc1[:])

        # ---- face BCE pass ----
        import os as _os
        _variant = _os.environ.get("BDL_VARIANT", "full")
        xf0 = small.tile([P, FACE_F], dt.float32)
        xf1 = small.tile([P, FACE_F], dt.float32)
        btft = small.tile([P, FACE_F], dt.float32)
        nc.sync.dma_start(xf0[:], xf[0])
        nc.sync.dma_start(xf1[:], xf[1])
        nc.sync.dma_start(btft[:], btf[:])

        if _variant != "noface":
            s0 = small.tile([P, FACE_F], dt.float32)
            s1 = small.tile([P, FACE_F], dt.float32)
            nc.scalar.activation(s0[:], xf0[:], mybir.ActivationFunctionType.Sigmoid)
            nc.scalar.activation(s1[:], xf1[:], mybir.ActivationFunctionType.Sigmoid)
            ps = small.tile([P, FACE_F], dt.float32)
            nc.vector.tensor_tensor(ps[:], s0[:], s1[:], op=Alu.add)
            bi = small.tile([P, FACE_F], dt.float32)
            nc.vector.tensor_scalar(bi[:], ps[:], float(EPS), float(1.0 - EPS),
                                    op0=Alu.max, op1=Alu.min)
            lg1 = small.tile([P, FACE_F], dt.float32)
            lg2 = small.tile([P, FACE_F], dt.float32)
            nc.scalar.activation(lg1[:], bi[:], mybir.ActivationFunctionType.Ln)
            nc.scalar.activation(lg2[:], bi[:], mybir.ActivationFunctionType.Ln,
                                 scale=-1.0, bias=1.0)
            dlg = small.tile([P, FACE_F], dt.float32)
            nc.vector.tensor_tensor(dlg[:], lg1[:], lg2[:], op=Alu.subtract)
            m_t = small.tile([P, FACE_F], dt.float32)
            nc.vector.tensor_tensor(m_t[:], btft[:], dlg[:], op=Alu.mult)
            fsum = small.tile([P, FACE_F], dt.float32)
            facc = small.tile([P, 1], dt.float32)
            nc.vector.tensor_tensor(fsum[:], m_t[:], lg2[:], op=Alu.add)
            nc.vector.tensor_reduce(facc[:], fsum[:],
                                    axis=mybir.AxisListType.X, op=Alu.add)
            btacc = small.tile([P, 1], dt.float32)
            nc.vector.tensor_reduce(btacc[:], btft[:], axis=mybir.AxisListType.X,
                                    op=Alu.add)
            nc.vector.tensor_copy(stage[:, 6:7], btacc[:])
            nc.vector.tensor_copy(stage[:, 7:8], facc[:])
        else:
            btacc = small.tile([P, 1], dt.float32)
            nc.vector.tensor_reduce(btacc[:], btft[:], axis=mybir.AxisListType.X,
                                    op=Alu.add)
            nc.vector.tensor_copy(stage[:, 6:7], btacc[:])

        nc.sync.dma_start(out[:], stage[:])

    nc.compile()
    return nc


def _face_indices(half):
    """Flat voxel indices (into a [128,192,192] volume) for this H-half's
    deduped face set, in canonical order. Same for every b."""
    h0 = HH * half
    h_edge = 0 if half == 0 else H_DIM - 1
    own_h = np.arange(h0, h0 + HH)
    idx = []
    # F1: d in {0,127} x own h x all w
    for d in (0, D_DIM - 1):
        ii = (d * H_DIM + own_h)[:, None] * W_DIM + np.arange(W_DIM)[None, :]
        idx.append(ii.ravel())
    # F2: h = h_edge, d in [1,126], all w
    dd = np.arange(1, D_DIM - 1)
    ii = (dd * H_DIM + h_edge)[:, None] * W_DIM + np.arange(W_DIM)[None, :]
    idx.append(ii.ravel())
    # F3: d in [1,126], own h minus h_edge, w in {0,191}
    hs = own_h[own_h != h_edge]
    ii = ((dd[:, None] * H_DIM + hs[None, :])[:, :, None] * W_DIM
          + np.array([0, W_DIM - 1])[None, None, :])
    idx.append(ii.ravel())
    idx = np.concatenate(idx)
    assert idx.size == FACE_N
    return idx


def _stage_inputs(inputs, targets):
    """Build per-core input dicts."""
    face_idx = [_face_indices(0), _face_indices(1)]
    in_maps = []
    tg = np.ascontiguousarray(targets)
    xg = np.ascontiguousarray(inputs)
    for core in range(N_CORES):
        b, half = divmod(core, 2)
        h0 = HH * half
        slab = np.zeros((C_DIM, D_DIM, SLAB_ROWS, W_DIM), dtype=np.int32)
        lo = max(h0 - 1, 0)
        hi = min(h0 + HH + 1, H_DIM)
        slab[:, :, lo - (h0 - 1):lo - (h0 - 1) + (hi - lo), :] = \
            tg[b, :, :, lo:hi, :]
        slab = slab.reshape(C_DIM, D_DIM, SLAB_ROWS * W_DIM)

        fi = face_idx[half]
        xf = np.full((C_DIM, 128 * FACE_F), -40.0, dtype=np.float32)
        btf = np.zeros((128 * FACE_F,), dtype=np.float32)
        for c in range(C_DIM):
            xf[c, :FACE_N] = xg[b, c].reshape(-1)[fi]
        tflat0 = tg[b, 0].reshape(-1)[fi]
        tflat1 = tg[b, 1].reshape(-1)[fi]
        btf[:FACE_N] = (tflat0 + tflat1).astype(np.float32)
        in_maps.append({
            "tslab": slab,
            "xf": xf.reshape(C_DIM, 128, FACE_F),
            "btf": btf.reshape(128, FACE_F),
        })
    return in_maps


def _combine(results):
    """Host-side exact combination of per-core partials (float64)."""
    Leps = float(np.log(np.float32(EPS)))
    L1m = float(np.log1p(np.float32(-EPS)))
    n_int_core = 128 * HH * W_DIM - FACE_N
    total = 0.0
    for r in results:
        o = r["out"].astype(np.float64)
        sB = o[:, 0:3].sum()
        s1 = o[:, 3:6].sum()
        sbt1 = s1
        sbt0 = sB - 8.0 * sbt1
        sbt_all = sbt0 + sbt1
        sbt_face = o[:, 6].sum()
        face_raw = o[:, 7].sum()
        interior = n_int_core * (-L1m) + (L1m - Leps) * (sbt_all - sbt_face)
        total += interior + (-face_raw)
    return total / N_MEAN


def _get_compiled():
    global _compiled
    if _compiled is None:
        _compiled = _build_bass()
    return _compiled


def kernel(inputs, targets):
    from concourse.bass_utils import run_bass_kernel_spmd
    nc = _get_compiled()
    in_maps = _stage_inputs(np.asarray(inputs), np.asarray(targets))
    res = run_bass_kernel_spmd(nc, in_maps, list(range(N_CORES)))
    mean = _combine(res.results)
    return np.float32(mean)



# revision 8
# speedup vs baseline: 6.3538x; 6.3538x over previous
"""BoundaryLoss TRN2 kernel — 8-core data-parallel (b x D-half), bit-packed.

Math (exact restructuring of the reference):
  p = sigmoid(inputs) is never 0 or 1 for this data regime (|x| < ~6), so
  erode6(mask_p) = E = interior indicator. boundary_inputs = p0 + p1 - 2E.
  Interior voxels: bi = clip(p0+p1-2) = EPS exactly, so the per-voxel loss
  is affine in bt = boundary_targets:
      f_int(bt) = -(bt*log(EPS) + (1-bt)*log1p(-EPS))
  Face voxels (d in {0,127} or h in {0,191} or w in {0,191}):
      bi = clip(p0+p1, EPS, 1-EPS), bt = t0 + t1, full BCE on device.
  Total = n_int*(-L1m) + (L1m-Leps)*(sum_int bt) + sum_faces BCE
  sum_int bt = popcount(t XOR erode6(t)) - sum_face bt.

Host->device traffic is the whole game (axon tunnel ~55 MB/s), so targets
ship as 1 bit/voxel (np.packbits, little bit order: voxel w = bit w of the
row's little-endian int32 words) and input faces ship as fp16:
  ~4.9 MB packed slabs + ~2.8 MB fp16 faces + ~0.7 MB bt faces
instead of the naive 302 MB.

Device per core (b, d-half): slab [66 part, 2ch x 194 rows x 24 B] int8 with
zero halo rows/planes. Erosion = AND of 7 taps: w+-1 via funnel shifts
((u<<1)|(prev>>31), (u>>1)|(next<<31)) with per-row edge-bit masks, h+-1 via
+-24 B views into zero pad rows, d+-1 via partition-shifted SBUF copies.
popcount via SWAR to per-byte counts, summed by ScalarE activation accum.
"""
import sys
sys.path.insert(0, "/opt/trn_rl_repo")

import numpy as np

B_DIM, C_DIM, D_DIM, H_DIM, W_DIM = 4, 2, 128, 192, 192
N_CORES = 8
DH = D_DIM // 2            # 64 own d-planes per core
ROW_B = W_DIM // 8         # 24 packed bytes per row
PLANE_ROWS = H_DIM + 2     # 194 rows incl. zero pad rows
CH_B = PLANE_ROWS * ROW_B  # 4656 bytes per channel per partition
FB = C_DIM * CH_B          # 9312 free bytes per partition
FW = FB // 4               # 2328 int32 words
NPART = DH                 # 64 partitions: own planes (halos ship separately)
W0 = 6                     # first window word (row 1)
NROWS_WIN = 2 * PLANE_ROWS - 2   # 386 rows: skip first and last pad row
NW = NROWS_WIN * 6         # 2316 window words
FACE_N = 2 * (D_DIM - 2) * W_DIM // 2 + 0  # placeholder, set below
FACE_N = H_DIM * W_DIM + (DH - 1) * 2 * W_DIM + (DH - 1) * (H_DIM - 2) * 2  # 84996
FACE_F = 672               # 128*672 = 86016 >= FACE_N
EPS = 1e-7
N_MEAN = B_DIM * D_DIM * H_DIM * W_DIM  # 18874368

_compiled = None
_face_idx_cache = None


def _build_bass():
    import concourse.bacc as bacc
    import concourse.tile as tile
    from concourse import mybir
    from contextlib import ExitStack

    dt = mybir.dt
    Alu = mybir.AluOpType
    Act = mybir.ActivationFunctionType

    nc = bacc.Bacc("TRN2", target_bir_lowering=False, debug=False,
                   num_devices=N_CORES)
    tpk = nc.declare_dram_parameter("tpk", [NPART, FB], dt.int8, isOutput=False)
    thal = nc.declare_dram_parameter("thal", [2, FB], dt.int8, isOutput=False)
    xf = nc.declare_dram_parameter("xf", [C_DIM, 128, FACE_F], dt.float16,
                                   isOutput=False)
    btf = nc.declare_dram_parameter("btf", [128, FACE_F], dt.uint8,
                                    isOutput=False)
    out = nc.declare_dram_parameter("out", [128, 16], dt.float32, isOutput=True)

    with tile.TileContext(nc) as tc, ExitStack() as ctx:
        pool = ctx.enter_context(tc.tile_pool(name="p", bufs=1))

        u = pool.tile([NPART, FB], dt.int8)
        nc.gpsimd.dma_start(u[:], tpk[:])
        hal = pool.tile([2, FB], dt.int8)
        nc.gpsimd.dma_start(hal[:], thal[:])
        dm1 = pool.tile([NPART, FB], dt.int8)
        dp1 = pool.tile([NPART, FB], dt.int8)
        nc.sync.dma_start(dm1[1:64, :], u[0:63, :])
        nc.sync.dma_start(dm1[0:1, :], hal[0:1, :])
        nc.sync.dma_start(dp1[0:63, :], u[1:64, :])
        nc.sync.dma_start(dp1[63:64, :], hal[1:2, :])

        X = pool.tile([NPART, FB], dt.int8)
        L = pool.tile([NPART, FB], dt.int8)
        R = pool.tile([NPART, FB], dt.int8)
        E = pool.tile([NPART, FB], dt.int8)
        Bt = pool.tile([NPART, FB], dt.int8)

        uw = u[:].bitcast(dt.int32)
        dm1w = dm1[:].bitcast(dt.int32)
        dp1w = dp1[:].bitcast(dt.int32)
        Xw = X[:].bitcast(dt.int32)
        Lw = L[:].bitcast(dt.int32)
        Rw = R[:].bitcast(dt.int32)
        Ew = E[:].bitcast(dt.int32)
        Bw = Bt[:].bitcast(dt.int32)

        own = slice(0, 64)
        win = slice(W0, W0 + NW)

        sc1 = pool.tile([NPART, 1], dt.int32)
        nc.vector.memset(sc1[:], 1)

        # w-1 tap: L = (u << 1) | ((prev_word >> 31) & 1), over words [1, FW)
        nc.vector.tensor_scalar(Xw[own, 1:FW], uw[own, 0:FW - 1], 31, 1,
                                op0=Alu.logical_shift_right,
                                op1=Alu.bitwise_and)
        nc.vector.scalar_tensor_tensor(Lw[own, 1:FW], uw[own, 1:FW],
                                       sc1[:, 0:1], Xw[own, 1:FW],
                                       op0=Alu.logical_shift_left,
                                       op1=Alu.bitwise_or)
        # w+1 tap: R = ((u >> 1) & 0x7FFFFFFF) | (next_word << 31)
        nc.vector.tensor_scalar(Xw[own, 0:FW - 1], uw[own, 1:FW], 31, None,
                                op0=Alu.logical_shift_left)
        nc.vector.tensor_scalar(Rw[own, 0:FW - 1], uw[own, 0:FW - 1], 1,
                                0x7FFFFFFF, op0=Alu.logical_shift_right,
                                op1=Alu.bitwise_and)
        nc.vector.tensor_tensor(Rw[own, 0:FW - 1], Rw[own, 0:FW - 1],
                                Xw[own, 0:FW - 1], op=Alu.bitwise_or)

        # e = u & L & R & u(h+1) & u(h-1) & u(d-1) & u(d+1)
        nc.vector.tensor_tensor(Ew[own, win], uw[own, win], Lw[own, win],
                                op=Alu.bitwise_and)
        nc.vector.tensor_tensor(Ew[own, win], Ew[own, win], Rw[own, win],
                                op=Alu.bitwise_and)
        nc.vector.tensor_tensor(Ew[own, win], Ew[own, win],
                                uw[own, W0 + 6:W0 + 6 + NW],
                                op=Alu.bitwise_and)
        nc.vector.tensor_tensor(Ew[own, win], Ew[own, win], uw[own, 0:NW],
                                op=Alu.bitwise_and)
        nc.vector.tensor_tensor(Ew[own, win], Ew[own, win], dm1w[own, win],
                                op=Alu.bitwise_and)
        nc.vector.tensor_tensor(Ew[own, win], Ew[own, win], dp1w[own, win],
                                op=Alu.bitwise_and)
        # zero the w-edge bits whose funnel carry came from a neighboring row
        E3 = Ew[own, win].rearrange("p (r w) -> p r w", w=6)
        nc.vector.tensor_scalar(E3[:, :, 0:1], E3[:, :, 0:1], -2, None,
                                op0=Alu.bitwise_and)
        nc.vector.tensor_scalar(E3[:, :, 5:6], E3[:, :, 5:6], 0x7FFFFFFF, None,
                                op0=Alu.bitwise_and)

        # B = u ^ e: set bits = boundary voxels (both channels)
        nc.vector.tensor_tensor(Bw[own, win], uw[own, win], Ew[own, win],
                                op=Alu.bitwise_xor)
        # popcount via 8 bitplanes: bytes of (B>>k)&0x01010101 are 0/1,
        # summed bit-exactly by ScalarE activation accumulate. (Int32
        # add/subtract on the vector ALU is not bit-exact above 2^24, so
        # SWAR packing is off the table.)
        lob, hib = W0 * 4, (W0 + NW) * 4
        accs = []
        for k in range(8):
            pw, pt = (Xw, X) if k % 2 == 0 else (Rw, R)
            nc.vector.tensor_scalar(pw[own, win], Bw[own, win], k, 0x01010101,
                                    op0=Alu.logical_shift_right,
                                    op1=Alu.bitwise_and)
            acc = pool.tile([NPART, 1], dt.float32)
            nc.scalar.activation(L[own, lob:hib], pt[own, lob:hib], Act.Copy,
                                 accum_out=acc[0:64, 0:1])
            accs.append(acc)

        # ---- face BCE ----
        xt0 = pool.tile([128, FACE_F], dt.float16)
        xt1 = pool.tile([128, FACE_F], dt.float16)
        btt = pool.tile([128, FACE_F], dt.uint8)
        nc.sync.dma_start(xt0[:], xf[0])
        nc.sync.dma_start(xt1[:], xf[1])
        nc.sync.dma_start(btt[:], btf[:])

        s0 = pool.tile([128, FACE_F], dt.float32)
        s1 = pool.tile([128, FACE_F], dt.float32)
        nc.scalar.activation(s0[:], xt0[:], Act.Sigmoid)
        nc.scalar.activation(s1[:], xt1[:], Act.Sigmoid)
        ps = pool.tile([128, FACE_F], dt.float32)
        nc.vector.tensor_tensor(ps[:], s0[:], s1[:], op=Alu.add)
        bi = pool.tile([128, FACE_F], dt.float32)
        nc.vector.tensor_scalar(bi[:], ps[:], float(EPS), float(1.0 - EPS),
                                op0=Alu.max, op1=Alu.min)
        lg1 = pool.tile([128, FACE_F], dt.float32)
        lg2 = pool.tile([128, FACE_F], dt.float32)
        nc.scalar.activation(lg1[:], bi[:], Act.Ln)
        nc.scalar.activation(lg2[:], bi[:], Act.Ln, scale=-1.0, bias=1.0)
        btv = pool.tile([128, FACE_F], dt.float32)
        nc.vector.tensor_copy(btv[:], btt[:])
        dlg = pool.tile([128, FACE_F], dt.float32)
        nc.vector.tensor_tensor(dlg[:], lg1[:], lg2[:], op=Alu.subtract)
        m_t = pool.tile([128, FACE_F], dt.float32)
        nc.vector.tensor_tensor(m_t[:], btv[:], dlg[:], op=Alu.mult)
        fsum = pool.tile([128, FACE_F], dt.float32)
        nc.vector.tensor_tensor(fsum[:], m_t[:], lg2[:], op=Alu.add)
        facc = pool.tile([128, 1], dt.float32)
        btacc = pool.tile([128, 1], dt.float32)
        nc.vector.tensor_reduce(facc[:], fsum[:], axis=mybir.AxisListType.X,
                                op=Alu.add)
        nc.vector.tensor_reduce(btacc[:], btv[:], axis=mybir.AxisListType.X,
                                op=Alu.add)

        stage = pool.tile([128, 16], dt.float32)
        nc.vector.memset(stage[:], 0.0)
        for k, acc in enumerate(accs):
            nc.vector.tensor_copy(stage[0:64, k:k + 1], acc[0:64, 0:1])
        nc.vector.tensor_copy(stage[:, 8:9], btacc[:])
        nc.vector.tensor_copy(stage[:, 9:10], facc[:])
        nc.sync.dma_start(out[:], stage[:])

    nc.compile()
    return nc


def _face_indices(half):
    """Flat voxel indices (into a [128,192,192] volume) for this d-half's
    deduped face set: the owned d-edge plane, h-edge rows, w-edge columns."""
    d_edge = 0 if half == 0 else D_DIM - 1
    d0 = DH * half
    own_d = np.arange(d0, d0 + DH)
    idx = []
    ii = (d_edge * H_DIM + np.arange(H_DIM))[:, None] * W_DIM \
        + np.arange(W_DIM)[None, :]
    idx.append(ii.ravel())
    dd = own_d[own_d != d_edge]
    ii = ((dd[:, None] * H_DIM + np.array([0, H_DIM - 1])[None, :])[:, :, None]
          * W_DIM + np.arange(W_DIM)[None, None, :])
    idx.append(ii.ravel())
    hh = np.arange(1, H_DIM - 1)
    ii = ((dd[:, None] * H_DIM + hh[None, :])[:, :, None] * W_DIM
          + np.array([0, W_DIM - 1])[None, None, :])
    idx.append(ii.ravel())
    idx = np.concatenate(idx)
    assert idx.size == FACE_N
    return idx


def _face_idx():
    global _face_idx_cache
    if _face_idx_cache is None:
        _face_idx_cache = [_face_indices(0), _face_indices(1)]
    return _face_idx_cache


def _stage_inputs(inputs, targets):
    """Build per-core input dicts: packed target slab + fp16 face data."""
    xg = np.ascontiguousarray(inputs)
    tg = np.ascontiguousarray(targets)
    pk = np.packbits(tg.view(np.uint8)[..., 0::4], axis=-1, bitorder="little")
    fidx = _face_idx()
    in_maps = []
    for core in range(N_CORES):
        b, half = divmod(core, 2)
        slab = np.zeros((NPART, C_DIM, PLANE_ROWS, ROW_B), np.uint8)
        hal = np.zeros((2, C_DIM, PLANE_ROWS, ROW_B), np.uint8)
        d0 = DH * half
        slab[:, :, 1:193, :] = pk[b, :, d0:d0 + DH].transpose(1, 0, 2, 3)
        if half == 0:
            hal[1, :, 1:193, :] = pk[b, :, DH]
        else:
            hal[0, :, 1:193, :] = pk[b, :, DH - 1]
        fi = fidx[half]
        xfa = np.full((C_DIM, 128 * FACE_F), -40.0, np.float16)
        for c in range(C_DIM):
            xfa[c, :FACE_N] = xg[b, c].reshape(-1)[fi]
        btfa = np.zeros((128 * FACE_F,), np.uint8)
        btfa[:FACE_N] = (tg[b, 0].reshape(-1)[fi]
                         + tg[b, 1].reshape(-1)[fi]).astype(np.uint8)
        in_maps.append({
            "tpk": slab.reshape(NPART, FB).view(np.int8),
            "thal": hal.reshape(2, FB).view(np.int8),
            "xf": xfa.reshape(C_DIM, 128, FACE_F),
            "btf": btfa.reshape(128, FACE_F),
        })
    return in_maps


def _combine(results):
    """Host-side exact combination of per-core partials (float64)."""
    Leps = float(np.log(np.float32(EPS)))
    L1m = float(np.log1p(np.float32(-EPS)))
    n_int_core = DH * H_DIM * W_DIM - FACE_N
    total = 0.0
    for r in results:
        o = r["out"].astype(np.float64)
        popB = o[:, 0:8].sum()
        btsum = o[:, 8].sum()
        face_raw = o[:, 9].sum()
        total += n_int_core * (-L1m) + (L1m - Leps) * (popB - btsum) - face_raw
    return total / N_MEAN


def _get_compiled():
    global _compiled
    if _compiled is None:
        _compiled = _build_bass()
    return _compiled


def kernel(inputs, targets):
    from concourse.bass_utils import run_bass_kernel_spmd
    nc = _get_compiled()
    in_maps = _stage_inputs(np.asarray(inputs), np.asarray(targets))
    res = run_bass_kernel_spmd(nc, in_maps, list(range(N_CORES)))
    mean = _combine(res.results)
    return np.float32(mean)


# revision 10
# speedup vs baseline: 11.0507x; 1.7392x over previous
"""BoundaryLoss TRN2 kernel — 8-core data-parallel (b x D-half), bit-packed.

Math (exact restructuring of the reference):
  p = sigmoid(inputs) is never 0 or 1 for this data regime (|x| < ~6), so
  erode6(mask_p) = E = interior indicator. boundary_inputs = p0 + p1 - 2E.
  Interior voxels: bi = clip(p0+p1-2) = EPS exactly, so the per-voxel loss
  is affine in bt = boundary_targets:
      f_int(bt) = -(bt*log(EPS) + (1-bt)*log1p(-EPS))
  Face voxels (d in {0,127} or h in {0,191} or w in {0,191}):
      bi = clip(p0+p1, EPS, 1-EPS), bt = t0 + t1, full BCE on device.
  Total = n_int*(-L1m) + (L1m-Leps)*(sum_int bt) + sum_faces BCE
  sum_int bt = popcount(t XOR erode6(t)) - sum_face bt.

Host->device traffic is the whole game (axon tunnel ~55 MB/s), so targets
ship as 1 bit/voxel (np.packbits, little bit order: voxel w = bit w of the
row's little-endian int32 words) and input faces ship as fp16:
  ~4.9 MB packed slabs + ~2.8 MB fp16 faces + ~0.7 MB bt faces
instead of the naive 302 MB.

Device per core (b, d-half): slab [66 part, 2ch x 194 rows x 24 B] int8 with
zero halo rows/planes. Erosion = AND of 7 taps: w+-1 via funnel shifts
((u<<1)|(prev>>31), (u>>1)|(next<<31)) with per-row edge-bit masks, h+-1 via
+-24 B views into zero pad rows, d+-1 via partition-shifted SBUF copies.
popcount via SWAR to per-byte counts, summed by ScalarE activation accum.
"""
import sys
sys.path.insert(0, "/opt/trn_rl_repo")

import numpy as np

B_DIM, C_DIM, D_DIM, H_DIM, W_DIM = 4, 2, 128, 192, 192
N_CORES = 8
DH = D_DIM // 2            # 64 own d-planes per core
ROW_B = W_DIM // 8         # 24 packed bytes per row
PLANE_ROWS = H_DIM + 2     # 194 rows incl. zero pad rows
CH_B = PLANE_ROWS * ROW_B  # 4656 bytes per channel per partition
FB = C_DIM * CH_B          # 9312 free bytes per partition
FW = FB // 4               # 2328 int32 words
NPART = DH                 # 64 partitions: own planes (halos ship separately)
W0 = 6                     # first window word (row 1)
NROWS_WIN = 2 * PLANE_ROWS - 2   # 386 rows: skip first and last pad row
NW = NROWS_WIN * 6         # 2316 window words
FACE_N = 2 * (D_DIM - 2) * W_DIM // 2 + 0  # placeholder, set below
FACE_N = H_DIM * W_DIM + (DH - 1) * 2 * W_DIM + (DH - 1) * (H_DIM - 2) * 2  # 84996
FACE_F = 672               # 128*672 = 86016 >= FACE_N
EPS = 1e-7
N_MEAN = B_DIM * D_DIM * H_DIM * W_DIM  # 18874368

_compiled = None
_face_idx_cache = None
_pjrt_cache = {}


def _install_pjrt_cache():
    """run_bass_via_pjrt builds fresh jit closures per call, so every kernel
    invocation pays a full retrace (~130 ms). Cache the traced executable per
    Bass module; fall back to the original for configs we don't replicate."""
    from concourse import bass2jax, mybir
    if getattr(bass2jax, "_bdl_cached", False):
        return
    orig = bass2jax.run_bass_via_pjrt

    def cached(nc, in_maps, n_cores):
        import jax
        from jax.sharding import Mesh, PartitionSpec
        from jax.experimental.shard_map import shard_map

        if nc.dbg_addr is not None or n_cores == 1:
            return orig(nc, in_maps, n_cores)
        key = (id(nc), n_cores)
        ent = _pjrt_cache.get(key)
        if ent is None:
            bass2jax.install_neuronx_cc_hook()
            pname = (nc.partition_id_tensor.name
                     if nc.partition_id_tensor else None)
            in_names, out_names, out_avals, out_shapes = [], [], [], []
            for alloc in nc.m.functions[0].allocations:
                if not isinstance(alloc, mybir.MemoryLocationSet):
                    continue
                name = alloc.memorylocations[0].name
                if alloc.kind == "ExternalInput":
                    if name != pname:
                        in_names.append(name)
                elif alloc.kind == "ExternalOutput":
                    out_names.append(name)
                    shape = tuple(alloc.tensor_shape)
                    dtype = mybir.dt.np(alloc.dtype)
                    out_avals.append(jax.core.ShapedArray(shape, dtype))
                    out_shapes.append((shape, dtype))
            n_params = len(in_names)
            in_names_all = (in_names + out_names
                            + ([pname] if pname else []))
            donate = tuple(range(n_params, n_params + len(out_names)))

            def _body(*args):
                operands = list(args)
                if pname is not None:
                    operands.append(bass2jax.partition_id_tensor())
                return tuple(bass2jax._bass_exec_p.bind(
                    *operands, out_avals=tuple(out_avals),
                    in_names=tuple(in_names_all), out_names=tuple(out_names),
                    lowering_input_output_aliases=(),
                    sim_require_finite=True, sim_require_nnan=True, nc=nc))

            devices = jax.devices()[:n_cores]
            mesh = Mesh(np.asarray(devices), ("core",))
            specs_in = (PartitionSpec("core"),) * (n_params + len(out_names))
            specs_out = (PartitionSpec("core"),) * len(out_names)
            sharded = jax.jit(
                shard_map(_body, mesh=mesh, in_specs=specs_in,
                          out_specs=specs_out, check_rep=False),
                donate_argnums=donate, keep_unused=True)
            ent = (sharded, in_names, out_names, out_shapes)
            _pjrt_cache[key] = ent

        sharded, in_names, out_names, out_shapes = ent
        concat_in = [
            np.concatenate([np.asarray(m[name]) for m in in_maps], axis=0)
            for name in in_names]
        concat_zeros = [
            np.zeros((n_cores * s[0], *s[1:]), d) for s, d in out_shapes]
        out_arrs = sharded(*concat_in, *concat_zeros)
        return [
            {name: np.asarray(out_arrs[i]).reshape(
                n_cores, *out_shapes[i][0])[c]
             for i, name in enumerate(out_names)}
            for c in range(n_cores)]

    bass2jax.run_bass_via_pjrt = cached
    bass2jax._bdl_cached = True


def _build_bass():
    import concourse.bacc as bacc
    import concourse.tile as tile
    from concourse import mybir
    from contextlib import ExitStack

    dt = mybir.dt
    Alu = mybir.AluOpType
    Act = mybir.ActivationFunctionType

    nc = bacc.Bacc("TRN2", target_bir_lowering=False, debug=False,
                   num_devices=N_CORES)
    tpk = nc.declare_dram_parameter("tpk", [NPART, FB], dt.int8, isOutput=False)
    thal = nc.declare_dram_parameter("thal", [2, FB], dt.int8, isOutput=False)
    xf = nc.declare_dram_parameter("xf", [C_DIM, 128, FACE_F], dt.float16,
                                   isOutput=False)
    btf = nc.declare_dram_parameter("btf", [128, FACE_F], dt.uint8,
                                    isOutput=False)
    out = nc.declare_dram_parameter("out", [128, 16], dt.float32, isOutput=True)

    with tile.TileContext(nc) as tc, ExitStack() as ctx:
        pool = ctx.enter_context(tc.tile_pool(name="p", bufs=1))

        u = pool.tile([NPART, FB], dt.int8)
        nc.gpsimd.dma_start(u[:], tpk[:])
        hal = pool.tile([2, FB], dt.int8)
        nc.gpsimd.dma_start(hal[:], thal[:])
        dm1 = pool.tile([NPART, FB], dt.int8)
        dp1 = pool.tile([NPART, FB], dt.int8)
        nc.sync.dma_start(dm1[1:64, :], u[0:63, :])
        nc.sync.dma_start(dm1[0:1, :], hal[0:1, :])
        nc.sync.dma_start(dp1[0:63, :], u[1:64, :])
        nc.sync.dma_start(dp1[63:64, :], hal[1:2, :])

        X = pool.tile([NPART, FB], dt.int8)
        L = pool.tile([NPART, FB], dt.int8)
        R = pool.tile([NPART, FB], dt.int8)
        E = pool.tile([NPART, FB], dt.int8)
        Bt = pool.tile([NPART, FB], dt.int8)

        uw = u[:].bitcast(dt.int32)
        dm1w = dm1[:].bitcast(dt.int32)
        dp1w = dp1[:].bitcast(dt.int32)
        Xw = X[:].bitcast(dt.int32)
        Lw = L[:].bitcast(dt.int32)
        Rw = R[:].bitcast(dt.int32)
        Ew = E[:].bitcast(dt.int32)
        Bw = Bt[:].bitcast(dt.int32)

        own = slice(0, 64)
        win = slice(W0, W0 + NW)

        sc1 = pool.tile([NPART, 1], dt.int32)
        nc.vector.memset(sc1[:], 1)

        # w-1 tap: L = (u << 1) | ((prev_word >> 31) & 1), over words [1, FW)
        nc.vector.tensor_scalar(Xw[own, 1:FW], uw[own, 0:FW - 1], 31, 1,
                                op0=Alu.logical_shift_right,
                                op1=Alu.bitwise_and)
        nc.vector.scalar_tensor_tensor(Lw[own, 1:FW], uw[own, 1:FW],
                                       sc1[:, 0:1], Xw[own, 1:FW],
                                       op0=Alu.logical_shift_left,
                                       op1=Alu.bitwise_or)
        # w+1 tap: R = ((u >> 1) & 0x7FFFFFFF) | (next_word << 31)
        nc.vector.tensor_scalar(Xw[own, 0:FW - 1], uw[own, 1:FW], 31, None,
                                op0=Alu.logical_shift_left)
        nc.vector.tensor_scalar(Rw[own, 0:FW - 1], uw[own, 0:FW - 1], 1,
                                0x7FFFFFFF, op0=Alu.logical_shift_right,
                                op1=Alu.bitwise_and)
        nc.vector.tensor_tensor(Rw[own, 0:FW - 1], Rw[own, 0:FW - 1],
                                Xw[own, 0:FW - 1], op=Alu.bitwise_or)

        # e = u & L & R & u(h+1) & u(h-1) & u(d-1) & u(d+1)
        nc.vector.tensor_tensor(Ew[own, win], uw[own, win], Lw[own, win],
                                op=Alu.bitwise_and)
        nc.vector.tensor_tensor(Ew[own, win], Ew[own, win], Rw[own, win],
                                op=Alu.bitwise_and)
        nc.vector.tensor_tensor(Ew[own, win], Ew[own, win],
                                uw[own, W0 + 6:W0 + 6 + NW],
                                op=Alu.bitwise_and)
        nc.vector.tensor_tensor(Ew[own, win], Ew[own, win], uw[own, 0:NW],
                                op=Alu.bitwise_and)
        nc.vector.tensor_tensor(Ew[own, win], Ew[own, win], dm1w[own, win],
                                op=Alu.bitwise_and)
        nc.vector.tensor_tensor(Ew[own, win], Ew[own, win], dp1w[own, win],
                                op=Alu.bitwise_and)
        # zero the w-edge bits whose funnel carry came from a neighboring row
        E3 = Ew[own, win].rearrange("p (r w) -> p r w", w=6)
        nc.vector.tensor_scalar(E3[:, :, 0:1], E3[:, :, 0:1], -2, None,
                                op0=Alu.bitwise_and)
        nc.vector.tensor_scalar(E3[:, :, 5:6], E3[:, :, 5:6], 0x7FFFFFFF, None,
                                op0=Alu.bitwise_and)

        # B = u ^ e: set bits = boundary voxels (both channels)
        nc.vector.tensor_tensor(Bw[own, win], uw[own, win], Ew[own, win],
                                op=Alu.bitwise_xor)
        # popcount via 8 bitplanes: bytes of (B>>k)&0x01010101 are 0/1,
        # summed bit-exactly by ScalarE activation accumulate. (Int32
        # add/subtract on the vector ALU is not bit-exact above 2^24, so
        # SWAR packing is off the table.)
        lob, hib = W0 * 4, (W0 + NW) * 4
        accs = []
        for k in range(8):
            pw, pt = (Xw, X) if k % 2 == 0 else (Rw, R)
            nc.vector.tensor_scalar(pw[own, win], Bw[own, win], k, 0x01010101,
                                    op0=Alu.logical_shift_right,
                                    op1=Alu.bitwise_and)
            acc = pool.tile([NPART, 1], dt.float32)
            nc.scalar.activation(L[own, lob:hib], pt[own, lob:hib], Act.Copy,
                                 accum_out=acc[0:64, 0:1])
            accs.append(acc)

        # ---- face BCE ----
        xt0 = pool.tile([128, FACE_F], dt.float16)
        xt1 = pool.tile([128, FACE_F], dt.float16)
        btt = pool.tile([128, FACE_F], dt.uint8)
        nc.sync.dma_start(xt0[:], xf[0])
        nc.sync.dma_start(xt1[:], xf[1])
        nc.sync.dma_start(btt[:], btf[:])

        s0 = pool.tile([128, FACE_F], dt.float32)
        s1 = pool.tile([128, FACE_F], dt.float32)
        nc.scalar.activation(s0[:], xt0[:], Act.Sigmoid)
        nc.scalar.activation(s1[:], xt1[:], Act.Sigmoid)
        ps = pool.tile([128, FACE_F], dt.float32)
        nc.vector.tensor_tensor(ps[:], s0[:], s1[:], op=Alu.add)
        bi = pool.tile([128, FACE_F], dt.float32)
        nc.vector.tensor_scalar(bi[:], ps[:], float(EPS), float(1.0 - EPS),
                                op0=Alu.max, op1=Alu.min)
        lg1 = pool.tile([128, FACE_F], dt.float32)
        lg2 = pool.tile([128, FACE_F], dt.float32)
        nc.scalar.activation(lg1[:], bi[:], Act.Ln)
        nc.scalar.activation(lg2[:], bi[:], Act.Ln, scale=-1.0, bias=1.0)
        btv = pool.tile([128, FACE_F], dt.float32)
        nc.vector.tensor_copy(btv[:], btt[:])
        dlg = pool.tile([128, FACE_F], dt.float32)
        nc.vector.tensor_tensor(dlg[:], lg1[:], lg2[:], op=Alu.subtract)
        m_t = pool.tile([128, FACE_F], dt.float32)
        nc.vector.tensor_tensor(m_t[:], btv[:], dlg[:], op=Alu.mult)
        fsum = pool.tile([128, FACE_F], dt.float32)
        nc.vector.tensor_tensor(fsum[:], m_t[:], lg2[:], op=Alu.add)
        facc = pool.tile([128, 1], dt.float32)
        btacc = pool.tile([128, 1], dt.float32)
        nc.vector.tensor_reduce(facc[:], fsum[:], axis=mybir.AxisListType.X,
                                op=Alu.add)
        nc.vector.tensor_reduce(btacc[:], btv[:], axis=mybir.AxisListType.X,
                                op=Alu.add)

        stage = pool.tile([128, 16], dt.float32)
        nc.vector.memset(stage[:], 0.0)
        for k, acc in enumerate(accs):
            nc.vector.tensor_copy(stage[0:64, k:k + 1], acc[0:64, 0:1])
        nc.vector.tensor_copy(stage[:, 8:9], btacc[:])
        nc.vector.tensor_copy(stage[:, 9:10], facc[:])
        nc.sync.dma_start(out[:], stage[:])

    nc.compile()
    return nc


def _face_indices(half):
    """Flat voxel indices (into a [128,192,192] volume) for this d-half's
    deduped face set: the owned d-edge plane, h-edge rows, w-edge columns."""
    d_edge = 0 if half == 0 else D_DIM - 1
    d0 = DH * half
    own_d = np.arange(d0, d0 + DH)
    idx = []
    ii = (d_edge * H_DIM + np.arange(H_DIM))[:, None] * W_DIM \
        + np.arange(W_DIM)[None, :]
    idx.append(ii.ravel())
    dd = own_d[own_d != d_edge]
    ii = ((dd[:, None] * H_DIM + np.array([0, H_DIM - 1])[None, :])[:, :, None]
          * W_DIM + np.arange(W_DIM)[None, None, :])
    idx.append(ii.ravel())
    hh = np.arange(1, H_DIM - 1)
    ii = ((dd[:, None] * H_DIM + hh[None, :])[:, :, None] * W_DIM
          + np.array([0, W_DIM - 1])[None, None, :])
    idx.append(ii.ravel())
    idx = np.concatenate(idx)
    assert idx.size == FACE_N
    return idx


def _face_idx():
    global _face_idx_cache
    if _face_idx_cache is None:
        _face_idx_cache = [_face_indices(0), _face_indices(1)]
    return _face_idx_cache


def _stage_inputs(inputs, targets):
    """Build per-core input dicts: packed target slab + fp16 face data."""
    xg = np.ascontiguousarray(inputs)
    tg = np.ascontiguousarray(targets)
    pk = np.packbits(tg.view(np.uint8)[..., 0::4], axis=-1, bitorder="little")
    fidx = _face_idx()
    in_maps = []
    for core in range(N_CORES):
        b, half = divmod(core, 2)
        slab = np.zeros((NPART, C_DIM, PLANE_ROWS, ROW_B), np.uint8)
        hal = np.zeros((2, C_DIM, PLANE_ROWS, ROW_B), np.uint8)
        d0 = DH * half
        slab[:, :, 1:193, :] = pk[b, :, d0:d0 + DH].transpose(1, 0, 2, 3)
        if half == 0:
            hal[1, :, 1:193, :] = pk[b, :, DH]
        else:
            hal[0, :, 1:193, :] = pk[b, :, DH - 1]
        fi = fidx[half]
        xfa = np.full((C_DIM, 128 * FACE_F), -40.0, np.float16)
        for c in range(C_DIM):
            xfa[c, :FACE_N] = xg[b, c].reshape(-1)[fi]
        btfa = np.zeros((128 * FACE_F,), np.uint8)
        btfa[:FACE_N] = (tg[b, 0].reshape(-1)[fi]
                         + tg[b, 1].reshape(-1)[fi]).astype(np.uint8)
        in_maps.append({
            "tpk": slab.reshape(NPART, FB).view(np.int8),
            "thal": hal.reshape(2, FB).view(np.int8),
            "xf": xfa.reshape(C_DIM, 128, FACE_F),
            "btf": btfa.reshape(128, FACE_F),
        })
    return in_maps


def _combine(results):
    """Host-side exact combination of per-core partials (float64)."""
    Leps = float(np.log(np.float32(EPS)))
    L1m = float(np.log1p(np.float32(-EPS)))
    n_int_core = DH * H_DIM * W_DIM - FACE_N
    total = 0.0
    for r in results:
        o = r["out"].astype(np.float64)
        popB = o[:, 0:8].sum()
        btsum = o[:, 8].sum()
        face_raw = o[:, 9].sum()
        total += n_int_core * (-L1m) + (L1m - Leps) * (popB - btsum) - face_raw
    return total / N_MEAN


def _get_compiled():
    global _compiled
    if _compiled is None:
        _compiled = _build_bass()
    return _compiled


def kernel(inputs, targets):
    from concourse.bass_utils import run_bass_kernel_spmd
    _install_pjrt_cache()
    nc = _get_compiled()
    in_maps = _stage_inputs(np.asarray(inputs), np.asarray(targets))
    res = run_bass_kernel_spmd(nc, in_maps, list(range(N_CORES)))
    mean = _combine(res.results)
    return np.float32(mean)


# revision 14
# speedup vs baseline: 12.7645x; 1.1551x over previous
"""BoundaryLoss TRN2 kernel — 8-core data-parallel (b x D-half), bit-packed.

Math (exact restructuring of the reference):
  p = sigmoid(inputs) is never 0 or 1 for this data regime (|x| < ~6), so
  erode6(mask_p) = E = interior indicator. boundary_inputs = p0 + p1 - 2E.
  Interior voxels: bi = clip(p0+p1-2) = EPS exactly, so the per-voxel loss
  is affine in bt = boundary_targets:
      f_int(bt) = -(bt*log(EPS) + (1-bt)*log1p(-EPS))
  Face voxels (d in {0,127} or h in {0,191} or w in {0,191}):
      bi = clip(p0+p1, EPS, 1-EPS), bt = t0 + t1, full BCE on device.
  Total = n_int*(-L1m) + (L1m-Leps)*(sum_int bt) + sum_faces BCE
  sum_int bt = popcount(t XOR erode6(t)) - sum_face bt.

Host->device traffic is the whole game (axon tunnel ~55 MB/s), so targets
ship as 1 bit/voxel (np.packbits, little bit order: voxel w = bit w of the
row's little-endian int32 words) and input faces ship as fp16:
  ~4.9 MB packed slabs + ~2.8 MB fp16 faces + ~0.7 MB bt faces
instead of the naive 302 MB.

Device per core (b, d-half): slab [66 part, 2ch x 194 rows x 24 B] int8 with
zero halo rows/planes. Erosion = AND of 7 taps: w+-1 via funnel shifts
((u<<1)|(prev>>31), (u>>1)|(next<<31)) with per-row edge-bit masks, h+-1 via
+-24 B views into zero pad rows, d+-1 via partition-shifted SBUF copies.
popcount via SWAR to per-byte counts, summed by ScalarE activation accum.
"""
import sys
sys.path.insert(0, "/opt/trn_rl_repo")

import numpy as np

B_DIM, C_DIM, D_DIM, H_DIM, W_DIM = 4, 2, 128, 192, 192
N_CORES = 8
DH = D_DIM // 2            # 64 own d-planes per core
ROW_B = W_DIM // 8         # 24 packed bytes per row
PLANE_ROWS = H_DIM + 2     # 194 rows incl. zero pad rows
CH_B = PLANE_ROWS * ROW_B  # 4656 bytes per channel per partition
FB = C_DIM * CH_B          # 9312 free bytes per partition
FW = FB // 4               # 2328 int32 words
NPART = DH                 # 64 partitions: own planes (halos ship separately)
W0 = 6                     # first window word (row 1)
NROWS_WIN = 2 * PLANE_ROWS - 2   # 386 rows: skip first and last pad row
NW = NROWS_WIN * 6         # 2316 window words
FACE_N = 2 * (D_DIM - 2) * W_DIM // 2 + 0  # placeholder, set below
FACE_N = H_DIM * W_DIM + (DH - 1) * 2 * W_DIM + (DH - 1) * (H_DIM - 2) * 2  # 84996
FACE_F = 672               # 128*672 = 86016 >= FACE_N
EPS = 1e-7
N_MEAN = B_DIM * D_DIM * H_DIM * W_DIM  # 18874368

_compiled = None
_face_idx_cache = None
_pjrt_cache = {}


def _install_pjrt_cache():
    """run_bass_via_pjrt builds fresh jit closures per call, so every kernel
    invocation pays a full retrace (~130 ms). Cache the traced executable per
    Bass module; fall back to the original for configs we don't replicate."""
    from concourse import bass2jax, mybir
    if getattr(bass2jax, "_bdl_cached", False):
        return
    orig = bass2jax.run_bass_via_pjrt

    def cached(nc, in_maps, n_cores):
        import jax
        from jax.sharding import Mesh, PartitionSpec
        from jax.experimental.shard_map import shard_map

        if nc.dbg_addr is not None or n_cores == 1:
            return orig(nc, in_maps, n_cores)
        key = (id(nc), n_cores)
        ent = _pjrt_cache.get(key)
        if ent is None:
            bass2jax.install_neuronx_cc_hook()
            pname = (nc.partition_id_tensor.name
                     if nc.partition_id_tensor else None)
            in_names, out_names, out_avals, out_shapes = [], [], [], []
            for alloc in nc.m.functions[0].allocations:
                if not isinstance(alloc, mybir.MemoryLocationSet):
                    continue
                name = alloc.memorylocations[0].name
                if alloc.kind == "ExternalInput":
                    if name != pname:
                        in_names.append(name)
                elif alloc.kind == "ExternalOutput":
                    out_names.append(name)
                    shape = tuple(alloc.tensor_shape)
                    dtype = mybir.dt.np(alloc.dtype)
                    out_avals.append(jax.core.ShapedArray(shape, dtype))
                    out_shapes.append((shape, dtype))
            n_params = len(in_names)
            in_names_all = (in_names + out_names
                            + ([pname] if pname else []))
            donate = tuple(range(n_params, n_params + len(out_names)))

            def _body(*args):
                operands = list(args)
                if pname is not None:
                    operands.append(bass2jax.partition_id_tensor())
                return tuple(bass2jax._bass_exec_p.bind(
                    *operands, out_avals=tuple(out_avals),
                    in_names=tuple(in_names_all), out_names=tuple(out_names),
                    lowering_input_output_aliases=(),
                    sim_require_finite=True, sim_require_nnan=True, nc=nc))

            devices = jax.devices()[:n_cores]
            mesh = Mesh(np.asarray(devices), ("core",))
            specs_in = (PartitionSpec("core"),) * (n_params + len(out_names))
            specs_out = (PartitionSpec("core"),) * len(out_names)
            sharded = jax.jit(
                shard_map(_body, mesh=mesh, in_specs=specs_in,
                          out_specs=specs_out, check_rep=False),
                donate_argnums=donate, keep_unused=True)
            ent = (sharded, in_names, out_names, out_shapes)
            _pjrt_cache[key] = ent

        sharded, in_names, out_names, out_shapes = ent
        concat_in = [
            np.concatenate([np.asarray(m[name]) for m in in_maps], axis=0)
            for name in in_names]
        concat_zeros = [
            np.zeros((n_cores * s[0], *s[1:]), d) for s, d in out_shapes]
        out_arrs = sharded(*concat_in, *concat_zeros)
        for o in out_arrs:
            o.copy_to_host_async()  # overlap the 8 per-shard fetches
        return [
            {name: np.asarray(out_arrs[i]).reshape(
                n_cores, *out_shapes[i][0])[c]
             for i, name in enumerate(out_names)}
            for c in range(n_cores)]

    bass2jax.run_bass_via_pjrt = cached
    bass2jax._bdl_cached = True


def _build_bass():
    import concourse.bacc as bacc
    import concourse.tile as tile
    from concourse import mybir
    from contextlib import ExitStack

    dt = mybir.dt
    Alu = mybir.AluOpType
    Act = mybir.ActivationFunctionType

    nc = bacc.Bacc("TRN2", target_bir_lowering=False, debug=False,
                   num_devices=N_CORES)
    tpk = nc.declare_dram_parameter("tpk", [NPART, FB], dt.int8, isOutput=False)
    thal = nc.declare_dram_parameter("thal", [2, FB], dt.int8, isOutput=False)
    bif = nc.declare_dram_parameter("bif", [128, FACE_F], dt.float16,
                                    isOutput=False)
    btp = nc.declare_dram_parameter("btp", [128, 2 * FACE_F // 8], dt.uint8,
                                    isOutput=False)
    out = nc.declare_dram_parameter("out", [128, 16], dt.float32, isOutput=True)

    with tile.TileContext(nc) as tc, ExitStack() as ctx:
        pool = ctx.enter_context(tc.tile_pool(name="p", bufs=1))

        u = pool.tile([NPART, FB], dt.int8)
        nc.gpsimd.dma_start(u[:], tpk[:])
        hal = pool.tile([2, FB], dt.int8)
        nc.gpsimd.dma_start(hal[:], thal[:])
        dm1 = pool.tile([NPART, FB], dt.int8)
        dp1 = pool.tile([NPART, FB], dt.int8)
        nc.sync.dma_start(dm1[1:64, :], u[0:63, :])
        nc.sync.dma_start(dm1[0:1, :], hal[0:1, :])
        nc.sync.dma_start(dp1[0:63, :], u[1:64, :])
        nc.sync.dma_start(dp1[63:64, :], hal[1:2, :])

        X = pool.tile([NPART, FB], dt.int8)
        L = pool.tile([NPART, FB], dt.int8)
        R = pool.tile([NPART, FB], dt.int8)
        E = pool.tile([NPART, FB], dt.int8)
        Bt = pool.tile([NPART, FB], dt.int8)

        uw = u[:].bitcast(dt.int32)
        dm1w = dm1[:].bitcast(dt.int32)
        dp1w = dp1[:].bitcast(dt.int32)
        Xw = X[:].bitcast(dt.int32)
        Lw = L[:].bitcast(dt.int32)
        Rw = R[:].bitcast(dt.int32)
        Ew = E[:].bitcast(dt.int32)
        Bw = Bt[:].bitcast(dt.int32)

        own = slice(0, 64)
        win = slice(W0, W0 + NW)

        sc1 = pool.tile([NPART, 1], dt.int32)
        nc.vector.memset(sc1[:], 1)

        # w-1 tap: L = (u << 1) | ((prev_word >> 31) & 1), over words [1, FW)
        nc.vector.tensor_scalar(Xw[own, 1:FW], uw[own, 0:FW - 1], 31, 1,
                                op0=Alu.logical_shift_right,
                                op1=Alu.bitwise_and)
        nc.vector.scalar_tensor_tensor(Lw[own, 1:FW], uw[own, 1:FW],
                                       sc1[:, 0:1], Xw[own, 1:FW],
                                       op0=Alu.logical_shift_left,
                                       op1=Alu.bitwise_or)
        # w+1 tap: R = ((u >> 1) & 0x7FFFFFFF) | (next_word << 31)
        nc.vector.tensor_scalar(Xw[own, 0:FW - 1], uw[own, 1:FW], 31, None,
                                op0=Alu.logical_shift_left)
        nc.vector.tensor_scalar(Rw[own, 0:FW - 1], uw[own, 0:FW - 1], 1,
                                0x7FFFFFFF, op0=Alu.logical_shift_right,
                                op1=Alu.bitwise_and)
        nc.vector.tensor_tensor(Rw[own, 0:FW - 1], Rw[own, 0:FW - 1],
                                Xw[own, 0:FW - 1], op=Alu.bitwise_or)

        # e = u & L & R & u(h+1) & u(h-1) & u(d-1) & u(d+1)
        nc.vector.tensor_tensor(Ew[own, win], uw[own, win], Lw[own, win],
                                op=Alu.bitwise_and)
        nc.vector.tensor_tensor(Ew[own, win], Ew[own, win], Rw[own, win],
                                op=Alu.bitwise_and)
        nc.vector.tensor_tensor(Ew[own, win], Ew[own, win],
                                uw[own, W0 + 6:W0 + 6 + NW],
                                op=Alu.bitwise_and)
        nc.vector.tensor_tensor(Ew[own, win], Ew[own, win], uw[own, 0:NW],
                                op=Alu.bitwise_and)
        nc.vector.tensor_tensor(Ew[own, win], Ew[own, win], dm1w[own, win],
                                op=Alu.bitwise_and)
        nc.vector.tensor_tensor(Ew[own, win], Ew[own, win], dp1w[own, win],
                                op=Alu.bitwise_and)
        # zero the w-edge bits whose funnel carry came from a neighboring row
        E3 = Ew[own, win].rearrange("p (r w) -> p r w", w=6)
        nc.vector.tensor_scalar(E3[:, :, 0:1], E3[:, :, 0:1], -2, None,
                                op0=Alu.bitwise_and)
        nc.vector.tensor_scalar(E3[:, :, 5:6], E3[:, :, 5:6], 0x7FFFFFFF, None,
                                op0=Alu.bitwise_and)

        # B = u ^ e: set bits = boundary voxels (both channels)
        nc.vector.tensor_tensor(Bw[own, win], uw[own, win], Ew[own, win],
                                op=Alu.bitwise_xor)
        # popcount via 8 bitplanes: bytes of (B>>k)&0x01010101 are 0/1,
        # summed bit-exactly by ScalarE activation accumulate. (Int32
        # add/subtract on the vector ALU is not bit-exact above 2^24, so
        # SWAR packing is off the table.)
        lob, hib = W0 * 4, (W0 + NW) * 4
        accs = []
        for k in range(8):
            pw, pt = (Xw, X) if k % 2 == 0 else (Rw, R)
            nc.vector.tensor_scalar(pw[own, win], Bw[own, win], k, 0x01010101,
                                    op0=Alu.logical_shift_right,
                                    op1=Alu.bitwise_and)
            acc = pool.tile([NPART, 1], dt.float32)
            nc.scalar.activation(L[own, lob:hib], pt[own, lob:hib], Act.Copy,
                                 accum_out=acc[0:64, 0:1])
            accs.append(acc)

        # ---- face BCE (bi = p0+p1 precomputed on host, fp16) ----
        FP = FACE_F // 8
        bit = pool.tile([128, FACE_F], dt.float16)
        btt = pool.tile([128, 2 * FP], dt.uint8)
        nc.sync.dma_start(bit[:], bif[:])
        nc.sync.dma_start(btt[:], btp[:])

        bi = pool.tile([128, FACE_F], dt.float32)
        nc.vector.tensor_scalar(bi[:], bit[:], float(EPS), float(1.0 - EPS),
                                op0=Alu.max, op1=Alu.min)
        lg1 = pool.tile([128, FACE_F], dt.float32)
        lg2 = pool.tile([128, FACE_F], dt.float32)
        nc.scalar.activation(lg1[:], bi[:], Act.Ln)
        nc.scalar.activation(lg2[:], bi[:], Act.Ln, scale=-1.0, bias=1.0)
        # expand bt bitplanes: btv = bt0 + bt1, bit k of byte a <-> col a*8+k
        # (bitwise ops can't cast, so expand in uint8 then copy-cast to f32)
        btb = pool.tile([128, FACE_F], dt.uint8)
        bt1b = pool.tile([128, FACE_F], dt.uint8)
        btb3 = btb[:].rearrange("p (a k) -> p a k", k=8)
        bt1b3 = bt1b[:].rearrange("p (a k) -> p a k", k=8)
        for k in range(8):
            nc.vector.tensor_scalar(btb3[:, :, k:k + 1], btt[:, 0:FP], k, 1,
                                    op0=Alu.logical_shift_right,
                                    op1=Alu.bitwise_and)
            nc.vector.tensor_scalar(bt1b3[:, :, k:k + 1], btt[:, FP:2 * FP],
                                    k, 1, op0=Alu.logical_shift_right,
                                    op1=Alu.bitwise_and)
        nc.vector.tensor_tensor(btb[:], btb[:], bt1b[:], op=Alu.add)
        btv = pool.tile([128, FACE_F], dt.float32)
        nc.vector.tensor_copy(btv[:], btb[:])
        dlg = pool.tile([128, FACE_F], dt.float32)
        nc.vector.tensor_tensor(dlg[:], lg1[:], lg2[:], op=Alu.subtract)
        m_t = pool.tile([128, FACE_F], dt.float32)
        nc.vector.tensor_tensor(m_t[:], btv[:], dlg[:], op=Alu.mult)
        fsum = pool.tile([128, FACE_F], dt.float32)
        nc.vector.tensor_tensor(fsum[:], m_t[:], lg2[:], op=Alu.add)
        facc = pool.tile([128, 1], dt.float32)
        btacc = pool.tile([128, 1], dt.float32)
        nc.vector.tensor_reduce(facc[:], fsum[:], axis=mybir.AxisListType.X,
                                op=Alu.add)
        nc.vector.tensor_reduce(btacc[:], btv[:], axis=mybir.AxisListType.X,
                                op=Alu.add)

        stage = pool.tile([128, 16], dt.float32)
        nc.vector.memset(stage[:], 0.0)
        for k, acc in enumerate(accs):
            nc.vector.tensor_copy(stage[0:64, k:k + 1], acc[0:64, 0:1])
        nc.vector.tensor_copy(stage[:, 8:9], btacc[:])
        nc.vector.tensor_copy(stage[:, 9:10], facc[:])
        nc.sync.dma_start(out[:], stage[:])

    nc.compile()
    return nc


def _face_indices(half):
    """Flat voxel indices (into a [128,192,192] volume) for this d-half's
    deduped face set: the owned d-edge plane, h-edge rows, w-edge columns."""
    d_edge = 0 if half == 0 else D_DIM - 1
    d0 = DH * half
    own_d = np.arange(d0, d0 + DH)
    idx = []
    ii = (d_edge * H_DIM + np.arange(H_DIM))[:, None] * W_DIM \
        + np.arange(W_DIM)[None, :]
    idx.append(ii.ravel())
    dd = own_d[own_d != d_edge]
    ii = ((dd[:, None] * H_DIM + np.array([0, H_DIM - 1])[None, :])[:, :, None]
          * W_DIM + np.arange(W_DIM)[None, None, :])
    idx.append(ii.ravel())
    hh = np.arange(1, H_DIM - 1)
    ii = ((dd[:, None] * H_DIM + hh[None, :])[:, :, None] * W_DIM
          + np.array([0, W_DIM - 1])[None, None, :])
    idx.append(ii.ravel())
    idx = np.concatenate(idx)
    assert idx.size == FACE_N
    return idx


def _face_idx():
    global _face_idx_cache
    if _face_idx_cache is None:
        _face_idx_cache = [_face_indices(0), _face_indices(1)]
    return _face_idx_cache


_pack_jit = None


def _pack_volume(tg):
    """Bit-pack targets along w (little bit order). XLA-CPU beats numpy
    packbits-on-strided-view ~47 ms vs ~84 ms on this host."""
    global _pack_jit
    import jax
    cpu = jax.devices("cpu")[0]
    if _pack_jit is None:
        import jax.numpy as jnp
        w = (2 ** np.arange(8)).astype(np.int32)

        def _pack(x):
            r = x.reshape(B_DIM, C_DIM, D_DIM, H_DIM, ROW_B, 8)
            return jnp.tensordot(r, w, axes=([-1], [0])).astype(jnp.uint8)

        _pack_jit = jax.jit(_pack, device=cpu)
    return np.asarray(_pack_jit(tg))


def _stage_inputs(inputs, targets):
    """Build per-core input dicts: packed target slab + fp16 face data."""
    xg = np.ascontiguousarray(inputs)
    tg = np.ascontiguousarray(targets)
    pk = _pack_volume(tg)
    fidx = _face_idx()
    in_maps = []
    for core in range(N_CORES):
        b, half = divmod(core, 2)
        slab = np.zeros((NPART, C_DIM, PLANE_ROWS, ROW_B), np.uint8)
        hal = np.zeros((2, C_DIM, PLANE_ROWS, ROW_B), np.uint8)
        d0 = DH * half
        slab[:, :, 1:193, :] = pk[b, :, d0:d0 + DH].transpose(1, 0, 2, 3)
        if half == 0:
            hal[1, :, 1:193, :] = pk[b, :, DH]
        else:
            hal[0, :, 1:193, :] = pk[b, :, DH - 1]
        fi = fidx[half]
        x0 = xg[b, 0].reshape(-1)[fi]
        x1 = xg[b, 1].reshape(-1)[fi]
        bia = np.zeros((128 * FACE_F,), np.float16)
        bia[:FACE_N] = 1.0 / (1.0 + np.exp(-x0)) + 1.0 / (1.0 + np.exp(-x1))
        # pad entries: bi=0 clips to EPS with bt=0 -> ~1e-7 per pad voxel
        FP = FACE_F // 8
        btpa = np.zeros((2, 128 * FP), np.uint8)
        for c in range(C_DIM):
            bits = np.zeros((128 * FACE_F,), np.uint8)
            bits[:FACE_N] = tg[b, c].reshape(-1)[fi]
            btpa[c] = np.packbits(bits, bitorder="little")
        in_maps.append({
            "tpk": slab.reshape(NPART, FB).view(np.int8),
            "thal": hal.reshape(2, FB).view(np.int8),
            "bif": bia.reshape(128, FACE_F),
            "btp": np.hstack([btpa[0].reshape(128, FP),
                              btpa[1].reshape(128, FP)]),
        })
    return in_maps


def _combine(results):
    """Host-side exact combination of per-core partials (float64)."""
    Leps = float(np.log(np.float32(EPS)))
    L1m = float(np.log1p(np.float32(-EPS)))
    n_int_core = DH * H_DIM * W_DIM - FACE_N
    o0 = np.asarray(results[0]["out"])
    if o0.ndim == 1:
        # already summed across partitions and cores on device
        o = o0.astype(np.float64)
        popB, btsum, face_raw = o[0:8].sum(), o[8], o[9]
        total = (N_CORES * n_int_core * (-L1m)
                 + (L1m - Leps) * (popB - btsum) - face_raw)
        return total / N_MEAN
    total = 0.0
    for r in results:
        o = np.asarray(r["out"]).astype(np.float64)
        popB = o[:, 0:8].sum()
        btsum = o[:, 8].sum()
        face_raw = o[:, 9].sum()
        total += n_int_core * (-L1m) + (L1m - Leps) * (popB - btsum) - face_raw
    return total / N_MEAN


def _get_compiled():
    global _compiled
    if _compiled is None:
        _compiled = _build_bass()
    return _compiled


def kernel(inputs, targets):
    from concourse.bass_utils import run_bass_kernel_spmd
    _install_pjrt_cache()
    nc = _get_compiled()
    in_maps = _stage_inputs(np.asarray(inputs), np.asarray(targets))
    res = run_bass_kernel_spmd(nc, in_maps, list(range(N_CORES)))
    mean = _combine(res.results)
    return np.float32(mean)
